# revision 3
# baseline (speedup 1.0000x reference)
"""DualAttention Trainium2 kernel (nn_DualAttention_44341242364496), v3.

Reference math (per batch element, X = points[b], shape (N=4096, C=256)):
  q = X Wq^T + bq ; k = X Wk^T + bk          (N, 32)
  P = softmax(q k^T, axis=-1)                (N, N)
  v = X Wv^T + bv                            (N, 256)
  out_p = gamma * P v + X
  E = X^T X ; A = softmax(max_d(E) - E) == stable softmax(-E)
  out_c = gamma * (X A^T) + X
  out = gamma*(Pv) + gamma*(X A^T) + 2X

Distribution: 8 cores; core c handles batch b=c//2, query-row half h=c%2.

v3 structure (vs v2 baseline at ~135us):
 - gamma folded into Wv on host; gamma*bv enters once via a broadcast
   tile in the outc drain (P(v+bv)/denom = Pv/denom + bv).  vaug's
   ones-column comes from a one-time memset, so v chunks drain from
   PSUM with pure copies, split across ACT/DVE, two chunks per bank.
 - the +X residual of the channel branch is folded into the attnTg
   matrix (identity added to its diagonal blocks), so outc's drain is
   c_ps + gbv (one DVE add) instead of + x2 rows.
 - energy runs cb0 chunks DMA-paced, then replays cb1 from SBUF; the
   cb0 channel-softmax chain hides under the cb1 matmuls.
 - round-loop exp is split ACT (cols :EXPACT) / DVE Schraudolph bf16
   (cols EXPACT:), keeping the 64-round loop PE-bound.
 - x2T loads ride the vector DMA queue, x2 the sync queue (parallel
   issue); weights/biases on scalar/gpsimd queues.
 - zero-tile fp16 warmup matmuls open the PE HAM clock gate before the
   DMA-paced energy matmuls begin.  (fp32 warmups would hang HW.)
"""

import sys

sys.path.insert(0, "/opt/trn_rl_repo")

import numpy as np

import concourse.bass as bass  # noqa: F401
import concourse.mybir as mybir
import concourse.tile as tile
from concourse import bacc
from concourse.bass_utils import run_bass_kernel_spmd
from concourse.masks import make_identity

B, N, C = 4, 4096, 256
C8 = C // 8  # 32
NCORES = 8
HALF = N // 2  # 2048 query rows per core
NBLK = HALF // 128  # 16 output row blocks per core
KCH = N // 128  # 32 key chunks
P = 128
CV = C + 1  # 257: v channels + denominator ones-column

F32 = mybir.dt.float32
U16 = mybir.dt.uint16
BF16 = mybir.dt.bfloat16
F16 = mybir.dt.float16
AX = mybir.AxisListType
ALU = mybir.AluOpType
ACTF = mybir.ActivationFunctionType

# Schraudolph bf16 exp: u16 = round(s * 128*log2(e) + (16256 + c))
EXPA = 184.6649652337873
EXPB = 16250.5

_CACHE: dict = {}

import os

NWARM = int(os.environ.get("V3_NWARM", "10"))
EXPACT = int(os.environ.get("V3_EXPACT", "640"))  # cols on ACT; rest DVE

MQ = 512  # queries per macro block
NMM = HALF // MQ  # 4 macro blocks
RPM = KCH // 2  # 16 rounds per macro block


def _build_nc():
    nc = bacc.Bacc("TRN2", target_bir_lowering=False)

    x2T_d = nc.dram_tensor("x2T", [P, 2, N], F16, kind="ExternalInput")
    x2_d = nc.dram_tensor("x2", [P, KCH, C], F16, kind="ExternalInput")
    wqT_d = nc.dram_tensor("wqT", [P, 2, C8], F16, kind="ExternalInput")
    wkT_d = nc.dram_tensor("wkT", [P, 2, C8], F16, kind="ExternalInput")
    wvT_d = nc.dram_tensor("wvT", [P, 2, C], F16, kind="ExternalInput")
    bq_d = nc.dram_tensor("bqc4", [4 * C8, 1], F32, kind="ExternalInput")
    bk_d = nc.dram_tensor("bkc4", [4 * C8, 1], F32, kind="ExternalInput")
    gbv_d = nc.dram_tensor("gbv", [1, C], F32, kind="ExternalInput")
    gam_d = nc.dram_tensor("gam", [1, 1], F32, kind="ExternalInput")
    out_d = nc.dram_tensor("out_rows", [NBLK, P, C], BF16, kind="ExternalOutput")

    with tile.TileContext(nc) as tc:
        with (
            tc.tile_pool(name="singles", bufs=1) as singles,
            tc.tile_pool(name="persist", bufs=1) as persist,
            tc.tile_pool(name="pTp", bufs=3) as pTp,
            tc.tile_pool(name="sbout", bufs=8) as sbout,
            tc.tile_pool(name="small", bufs=16) as small,
            tc.tile_pool(name="psS", bufs=2, space="PSUM") as psS,
            tc.tile_pool(name="psO", bufs=4, space="PSUM") as psO,
        ):
            # ---------------- Phase A: loads & constants ----------------
            # HAM warmup: fp16 matmuls on a zeroed tile keep PE busy through
            # the 3.4us activity window so the clock gate opens (1.2 ->
            # 2.4 GHz) roughly when the first x2 chunk lands.
            if NWARM:
                wz = singles.tile([P, 512], F16, tag="wz")
                nc.gpsimd.memset(wz[:], 0.0)
                warm = [
                    psO.tile([P, 512], F32, tag="o", name=f"warm{i}")[:, :512]
                    for i in range(4)
                ]
                for i in range(NWARM):
                    nc.tensor.matmul(
                        warm[i % 4], wz[:, :P], wz[:], start=True, stop=True
                    )
            gb = singles.tile([P, 1], F32, tag="gb")
            nc.scalar.dma_start(gb[:], gam_d.ap().to_broadcast([P, 1]))
            wqT = singles.tile([P, 2, C8], F16, tag="wqT")
            nc.scalar.dma_start(wqT[:], wqT_d.ap())
            wkT = singles.tile([P, 2, C8], F16, tag="wkT")
            nc.scalar.dma_start(wkT[:], wkT_d.ap())
            wvT = singles.tile([P, 2, C], F16, tag="wvT")
            nc.scalar.dma_start(wvT[:], wvT_d.ap())
            bqc4 = singles.tile([4 * C8, 1], F32, tag="bqc4")
            nc.scalar.dma_start(bqc4[:], bq_d.ap())
            bkc4 = singles.tile([4 * C8, 1], F32, tag="bkc4")
            nc.scalar.dma_start(bkc4[:], bk_d.ap())
            x2 = persist.tile([P, KCH, C], F16, tag="x2")
            x2T = persist.tile([P, 2, N], F16, tag="x2T")
            for g in range(8):
                if g == 0:
                    nc.sync.dma_start(x2[:, 0:1, :], x2_d.ap()[:, 0:1, :])
                    nc.sync.dma_start(x2[:, 1:4, :], x2_d.ap()[:, 1:4, :])
                else:
                    nc.sync.dma_start(
                        x2[:, g * 4 : (g + 1) * 4, :],
                        x2_d.ap()[:, g * 4 : (g + 1) * 4, :],
                    )
                # x2T on the gpsimd queue: parallel issue with x2 on sync
                nc.gpsimd.dma_start(
                    x2T[:, :, g * 512 : (g + 1) * 512],
                    x2T_d.ap()[:, :, g * 512 : (g + 1) * 512],
                )
            gbv = singles.tile([P, C], F32, tag="gbv")
            nc.gpsimd.dma_start(
                gbv[:],
                bass.AP(tensor=gbv_d, offset=0, ap=[[0, P], [1, C]]),
            )
            ident = singles.tile([P, P], F32, tag="ident")
            make_identity(nc, ident[:])
            gh = singles.tile([P, 1], F32, tag="gh")
            nc.vector.tensor_scalar_mul(gh[:], gb[:], 0.5)
            # vaug ones-column (denominator source), set once
            vaug = persist.tile([P, KCH, CV], BF16, tag="vaug")
            nc.gpsimd.memset(vaug[:, :, C : C + 1], 1.0)

            # ------- Phase B: channel attention (E = X^T X, softmax) -------
            attn_n = singles.tile([P, 2, C], F32, tag="attn_n")
            attnTg = persist.tile([P, 2, C], F16, tag="attnTg")
            e_ps = [
                psO.tile([P, 512], F32, tag="o", name=f"e_{cb}")[:, :C]
                for cb in range(2)
            ]
            # kT2[32g:32g+32, 128t:128(t+1)] = k-dims of key chunk 4t+g
            kT2 = persist.tile([P, (KCH // 4) * P], F16, tag="kT2")
            # qT2: 4 replicated row strips of the core's 2048 query q-vals
            qT2 = persist.tile([P, HALF], F16, tag="qT2")

            def emit_energy(nk, cb):
                nc.tensor.matmul(
                    e_ps[cb],
                    x2[:, nk, cb * P : (cb + 1) * P],
                    x2[:, nk, :],
                    start=(nk == 0),
                    stop=(nk == KCH - 1),
                )

            def emit_v(nk):
                # two chunks nk, nk+1 share one PSUM bank; one drain copy
                vps = psO.tile([P, 512], F32, tag="o", name=f"v_{nk}")
                for half in range(2):
                    for cc in range(2):
                        nc.tensor.matmul(
                            vps[:, half * C : (half + 1) * C],
                            x2T[:, cc, (nk + half) * P : (nk + half + 1) * P],
                            wvT[:, cc, :],
                            start=(cc == 0),
                            stop=(cc == 1),
                        )
                src = vps[:].rearrange("a (two c) -> a two c", two=2, c=C)
                if (nk // 2) % 2 == 0:
                    nc.vector.tensor_copy(vaug[:, nk : nk + 2, :C], src)
                else:
                    nc.scalar.copy(vaug[:, nk : nk + 2, :C], src)

            def emit_k(quarter):
                # 4-way column-tiled: group g -> psum partitions 32g..32g+31,
                # keys of chunks {8q+g, 8q+4+g} (256 cols per group)
                kps = psS.tile([P, 1024], F32, tag="s", name=f"k_{quarter}")
                xr = [
                    x2T[:, cc, :].rearrange(
                        "a (t four p) -> a four t p", four=4, p=P
                    )
                    for cc in range(2)
                ]
                for cc in range(2):
                    for g in range(4):
                        nc.tensor.matmul(
                            kps[g * C8 : (g + 1) * C8, :256],
                            wkT[:, cc, :],
                            xr[cc][:, g, 2 * quarter : 2 * quarter + 2, :],
                            start=(cc == 0),
                            stop=(cc == 1),
                            tile_position=(0, g * C8),
                            skip_group_check=True,
                        )
                nc.scalar.activation(
                    kT2[:, quarter * 256 : (quarter + 1) * 256],
                    kps[:, :256],
                    ACTF.Identity,
                    bias=bkc4[:],
                )

            def emit_q(seg):
                # 4 replicated row strips of q via 4-way column tiling
                qps = psS.tile([P, 1024], F32, tag="s", name=f"q_{seg}")
                for cc in range(2):
                    for g in range(4):
                        nc.tensor.matmul(
                            qps[g * C8 : (g + 1) * C8, :512],
                            wqT[:, cc, :],
                            x2T[:, cc, seg * 512 : (seg + 1) * 512],
                            start=(cc == 0),
                            stop=(cc == 1),
                            tile_position=(0, g * C8),
                            skip_group_check=True,
                        )
                nc.scalar.activation(
                    qT2[:, seg * 512 : (seg + 1) * 512],
                    qps[:, :512],
                    ACTF.Identity,
                    bias=bqc4[:],
                )

            outc_sb = persist.tile([P, NBLK, C], F32, tag="outc_sb")

            def emit_outc(blk):
                # c_ps = gamma*attn_c-part@X + 2X (residual via attnTg ident)
                c_ps = psO.tile([P, 512], F32, tag="o", name=f"c_{blk}")[:, :C]
                for dd in range(2):
                    nc.tensor.matmul(
                        c_ps,
                        x2T[:, dd, blk * P : (blk + 1) * P],
                        attnTg[:, dd, :],
                        start=(dd == 0),
                        stop=(dd == 1),
                    )
                nc.vector.tensor_add(outc_sb[:, blk, :], c_ps, gbv[:])

            # Dense DMA-paced loop: as group g of x2/x2T lands, its cb0
            # energy chunks, v chunks, and (odd g) k/q quarter all emit.
            for g in range(8):
                for nk in range(4 * g, 4 * g + 4):
                    emit_energy(nk, 0)
                    if nk % 2 == 0:
                        emit_v(nk)
                if g % 2 == 1:
                    emit_k((g - 1) // 2)
                    emit_q((g - 1) // 2)

            def chain(cb):
                emin = small.tile([P, 1], F32, tag="sm", name=f"emin{cb}")
                nc.vector.tensor_reduce(
                    emin[:], e_ps[cb], axis=AX.X, op=ALU.min
                )
                emq = small.tile([P, 1], F32, tag="sm", name=f"emq{cb}")
                nc.vector.tensor_scalar_mul(emq[:], emin[:], 0.25)
                us = small.tile([P, 1], F32, tag="sm", name=f"us{cb}")
                nc.scalar.activation(
                    attn_n[:, cb, :],
                    e_ps[cb],
                    ACTF.Exp,
                    bias=emq[:],
                    scale=-0.25,
                    accum_out=us[:],
                )
                rc = small.tile([P, 1], F32, tag="sm", name=f"rc{cb}")
                nc.vector.reciprocal(rc[:], us[:])
                rcg = small.tile([P, 1], F32, tag="sm", name=f"rcg{cb}")
                nc.vector.tensor_mul(rcg[:], rc[:], gh[:])
                nc.vector.tensor_scalar_mul(
                    attn_n[:, cb, :], attn_n[:, cb, :], rcg[:]
                )

            def transposes(cb):
                # attn_n[:, cb, :] -> attnTg[:, :, cb*P:(cb+1)*P]; the
                # diagonal block (dd == cb) gains +ident so the outc matmul
                # also produces the 2X residual.
                for dd in range(2):
                    t_ps = psS.tile(
                        [P, 1024], F32, tag="s", name=f"t_{dd}{cb}"
                    )[:, :P]
                    nc.tensor.transpose(
                        t_ps,
                        attn_n[:, cb, dd * P : (dd + 1) * P],
                        ident[:],
                    )
                    dst = attnTg[:, dd, cb * P : (cb + 1) * P]
                    if dd == cb:
                        nc.vector.tensor_add(dst, t_ps, ident[:])
                    else:
                        nc.scalar.copy(dst, t_ps)

            # cb0 chain runs on ACT/DVE while PE replays cb1 energy from SBUF
            chain(0)
            for nk in range(KCH):
                emit_energy(nk, 1)
            transposes(0)
            chain(1)
            transposes(1)
            # all 16 outc blocks as one dense PE block before the attention
            # rounds (their DVE drains trail into the early rounds)
            for blk in range(NBLK):
                emit_outc(blk)

            # ---------------- Phase D: point attention ----------------
            # Round r covers key chunks 2r, 2r+1, which live in kT2 row
            # strips {0,1} (even r) or {2,3} (odd r): the two score matmuls
            # run concurrently in distinct PE row groups and write the two
            # full-bank halves of the double-buffered s_ps.  The LAG=2
            # software pipeline keeps PE free of mid-phase semaphore stalls.
            o_ps: dict = {}
            pT_t: dict = {}

            def emit_scores(m, r):
                gbase = 2 * (r % 2)
                tt = r // 2
                s_ps = psS.tile([P, 1024], F32, tag="s", name=f"s_{m}_{r}")
                for h in range(2):
                    g = gbase + h
                    nc.tensor.matmul(
                        s_ps[:, h * 512 : (h + 1) * 512],
                        kT2[g * C8 : (g + 1) * C8, tt * P : (tt + 1) * P],
                        qT2[g * C8 : (g + 1) * C8, m * 512 : (m + 1) * 512],
                        start=True,
                        stop=True,
                        tile_position=(g * C8, 0),
                        skip_group_check=True,
                    )
                pT = pTp.tile([P, 1024], BF16, tag="pT", name=f"p_{m}_{r}")
                nc.scalar.activation(
                    pT[:, :EXPACT], s_ps[:, :EXPACT], ACTF.Exp
                )
                nc.vector.tensor_scalar(
                    pT[:, EXPACT:1024].bitcast(U16),
                    s_ps[:, EXPACT:1024],
                    EXPA,
                    EXPB,
                    op0=ALU.mult,
                    op1=ALU.add,
                )
                pT_t[(m, r)] = pT

            def emit_pv(m, r):
                pT = pT_t.pop((m, r))
                for h in range(2):
                    kk = 2 * r + h
                    for j in range(4):
                        nc.tensor.matmul(
                            o_ps[(m, j)],
                            pT[:, h * 512 + j * P : h * 512 + (j + 1) * P],
                            vaug[:, kk, :],
                            start=(kk == 0),
                            stop=(kk == KCH - 1),
                        )

            def emit_epilogue(m, last=False):
                # PSUM-freeing copies first, back-to-back, so the next
                # macro's PV never waits on epilogue math; the final macro
                # has no successor, so it reads PSUM directly.
                osbs = []
                for j in range(4):
                    blk = m * 4 + j
                    ops = o_ps.pop((m, j))
                    if last:
                        osbs.append(ops)
                        continue
                    osb = sbout.tile([P, CV], F32, tag="acc", name=f"osb{blk}")
                    nc.vector.tensor_copy(osb[:], ops)
                    osbs.append(osb)
                for j in range(4):
                    blk = m * 4 + j
                    osb = osbs[j]
                    rq = small.tile([P, 1], F32, tag="sm", name=f"rq{blk}")
                    nc.vector.reciprocal(rq[:], osb[:, C : C + 1])
                    acc = sbout.tile([P, C], BF16, tag="acc", name=f"acc{blk}")
                    if last and j >= 2:
                        nc.scalar.mul(acc[:], osb[:, :C], rq[:])
                        nc.vector.tensor_add(acc[:], acc[:], outc_sb[:, blk, :])
                    else:
                        nc.vector.scalar_tensor_tensor(
                            acc[:],
                            osb[:, :C],
                            rq[:],
                            outc_sb[:, blk, :],
                            op0=ALU.mult,
                            op1=ALU.add,
                        )
                    nc.sync.dma_start(out_d.ap()[blk], acc[:])

            LAG = 2
            rounds = [(m, r) for m in range(NMM) for r in range(RPM)]
            for i, (m, r) in enumerate(rounds):
                if r == 0:
                    for j in range(4):
                        o_ps[(m, j)] = psO.tile(
                            [P, 512], F32, tag="o", name=f"o_{m}_{j}"
                        )[:, :CV]
                emit_scores(m, r)
                if i >= LAG:
                    pm, pr = rounds[i - LAG]
                    emit_pv(pm, pr)
                    if pr == RPM - 1:
                        emit_epilogue(pm)
            for i in range(len(rounds) - LAG, len(rounds)):
                pm, pr = rounds[i]
                emit_pv(pm, pr)
                if pr == RPM - 1:
                    emit_epilogue(pm, last=True)

    nc.compile()
    return nc


def _prep_core_inputs(points, core):
    b, h = core // 2, core % 2
    xb = np.asarray(points[b], dtype=np.float32)
    # own rows first, then the other half (key order is softmax-invariant
    # as long as kT and v use the same order, which they do)
    xp = np.concatenate([xb[h * HALF : (h + 1) * HALF], xb[(1 - h) * HALF : (2 - h) * HALF]])
    x2T = np.ascontiguousarray(
        (2.0 * xp).T.reshape(2, P, N).transpose(1, 0, 2)
    ).astype(np.float16)  # (128, 2, 4096)
    x2 = (2.0 * xp).reshape(KCH, P, C).transpose(1, 0, 2).astype(np.float16)
    return {"x2T": x2T, "x2": x2}


def _prep_shared_inputs(Wq, bq, Wk, bk, Wv, bv, gamma):
    g = float(np.asarray(gamma, np.float32).reshape(()))
    wqT = np.ascontiguousarray((0.5 * np.asarray(Wq, np.float32).T).reshape(2, P, C8).transpose(1, 0, 2)).astype(np.float16)
    wkT = np.ascontiguousarray((0.5 * np.asarray(Wk, np.float32).T).reshape(2, P, C8).transpose(1, 0, 2)).astype(np.float16)
    # gamma folded into Wv (x2 carries 2.0, hence 0.5)
    wvT_full = (0.5 * g) * np.asarray(Wv, np.float32).T  # (256, 256)
    wvT = np.ascontiguousarray(wvT_full.reshape(2, P, C).transpose(1, 0, 2)).astype(np.float16)
    return {
        "wqT": wqT,
        "wkT": wkT,
        "wvT": wvT,
        "bqc4": np.tile(np.asarray(bq, np.float32), 4).reshape(4 * C8, 1),
        "bkc4": np.tile(np.asarray(bk, np.float32), 4).reshape(4 * C8, 1),
        "gbv": (g * np.asarray(bv, np.float32)).reshape(1, C),
        "gam": np.asarray(gamma, np.float32).reshape(1, 1),
    }


def kernel(points, Wq, bq, Wk, bk, Wv, bv, gamma, **run_kwargs):
    if "nc" not in _CACHE:
        _CACHE["nc"] = _build_nc()
    nc = _CACHE["nc"]

    shared = _prep_shared_inputs(Wq, bq, Wk, bk, Wv, bv, gamma)
    in_maps = []
    for core in range(NCORES):
        m = dict(shared)
        m.update(_prep_core_inputs(points, core))
        in_maps.append(m)

    res = run_bass_kernel_spmd(
        nc, in_maps, core_ids=list(range(NCORES)), **run_kwargs
    )
    out = np.empty((B, N, C), dtype=np.float32)
    for core in range(NCORES):
        b, h = core // 2, core % 2
        out[b, h * HALF : (h + 1) * HALF] = (
            res.results[core]["out_rows"].reshape(HALF, C).astype(np.float32)
        )
    if run_kwargs:
        kernel.last_results = res  # expose profile info to test harness
    return out
